# revision 14
# baseline (speedup 1.0000x reference)
"""LIF (leaky integrate-and-fire) forward scan on 8 Trainium2 NeuronCores.

Reference recurrence (per element, scan over T):
    m_t = v_{t-1} * tau + x_t
    y_t = (m_t - v_th > 0) ? 1.0 : 0.0
    v_t = m_t * (1 - y_t)          # hard reset on spike

x: [T=16, B=32, C=128, H=32, W=32] f32.  Data-parallel over B: each core
gets B_loc=4 batches -> per-step tile [C=128 partitions, B_loc*H*W=4096].

Scaled formulation (tau = 0.5 is a power of two, so rescaling is EXACT
in fp32): host pre-scales X_t = x_t / tau^t; on-chip track M_t = m_t / tau^t:
    M_{t+1} = select(M_t <= 2^t, M_t, 0) + X_{t+1}
    y_t     = Sign(M_t * tau^t - 1)

The whole step update is ONE custom DVE op (registered at import into
concourse's custom-DVE table machinery; the uop program is written into
the per-NEFF DVE table, no firmware change).  The spike extraction is
one ACT op per step whose f32->uint8 output cast saturates Sign's -1
to 0, giving exact {0,1} bytes.  Per-core engine busy: DVE ~65us,
ACT ~60us, DMA ~111us -> the kernel is DMA(HBM)-bound.
"""

import sys

sys.path.insert(0, "/opt/trn_rl_repo")

from contextlib import ExitStack

import numpy as np

import concourse.bass as bass
import concourse.tile as tile
from concourse import bacc, mybir
from concourse.bass_utils import run_bass_kernel_spmd

# Hyperparameters (from the nn.Module)
V_TH = 1.0
TAU = 0.5

# Shapes (hardcoded per problem spec)
T, B, C, H, W = 16, 32, 128, 32, 32
N_CORES = 8
B_LOC = B // N_CORES           # 4 batches per core
S = H * W                      # 1024 spatial sites
FREE = B_LOC * S               # 4096 columns per step tile

DT = mybir.dt.float32

# ---- custom DVE op: out = select(in0 <= s0, in0, 0) + in1 ---------------- #


def _register_lif_op():
    from concourse import dve_ops
    from concourse.dve_spec import Spec, Src0, Src1, C0, Zero, select, lower
    from concourse.dve_uop import DveOpSpec

    name = "LIF_GATE_ADD_ANT"
    for op in dve_ops.OPS:
        if op.name == name:
            return op

    body = select(Src0 <= C0, Src0, Zero) + Src1
    spec = Spec(
        body=body,
        reference=lambda in0, in1, s0, s1, imm2: np.where(in0 <= s0, in0, 0.0)
        + in1,
    )
    row = dve_ops._CUSTOM_DVE_ROW_BASE + len(dve_ops.OPS)
    assert row < 0x20, "custom-DVE row field overflow"
    shas = {
        ver: DveOpSpec(
            name=name, opcode=row, uops=lower(spec, ver=ver), rd1_en=True
        ).sha(ver)
        for ver in ("v3", "v4")
    }
    op = dve_ops.DveOp(name, spec, subdim=False, uops_sha=shas)
    dve_ops.OPS.append(op)
    dve_ops._SUB_OPCODE_FOR_NAME[name] = row
    dve_ops.CUSTOM_DVE_SPECS[name] = spec
    return op


LIF_OP = _register_lif_op()


def build_kernel() -> bass.Bass:
    nc = bacc.Bacc(
        "TRN2", target_bir_lowering=False, debug=False, num_devices=N_CORES
    )
    x_d = nc.dram_tensor("x", [T, B_LOC, C, S], DT, kind="ExternalInput").ap()
    y_d = nc.dram_tensor(
        "y", [T, B_LOC, C, S], mybir.dt.uint8, kind="ExternalOutput"
    ).ap()

    # Const AP for the ACT bias (-1.0); activation bias needs [128,1] SBUF.
    _c = nc.alloc_sbuf_tensor(f"const-float32-{-V_TH}", [128, 1], DT)
    nc.gpsimd.memset(_c.ap(), -V_TH)
    nc.const_aps.aps[(DT, -V_TH)] = _c.ap()
    nc.all_engine_barrier()

    HB = B_LOC // 2  # 2 independent column-half chains (finer pipelining)
    with ExitStack() as ctx:
        tc = ctx.enter_context(tile.TileContext(nc))
        x_pools = [
            ctx.enter_context(tc.tile_pool(name=f"x{h}", bufs=5))
            for h in range(2)
        ]
        m_pools = [
            ctx.enter_context(tc.tile_pool(name=f"m{h}", bufs=3))
            for h in range(2)
        ]
        y_pools = [
            ctx.enter_context(tc.tile_pool(name=f"y{h}", bufs=4))
            for h in range(2)
        ]

        m_cur = [None, None]
        for t in range(T):
            for h in range(2):
                b0 = h * HB
                xt = x_pools[h].tile([C, HB, S], DT, tag=f"x{h}")
                nc.sync.dma_start(
                    out=xt[:],
                    in_=x_d[t, b0 : b0 + HB].rearrange("b c s -> c b s"),
                )

                if t == 0:
                    m = xt[:]
                else:
                    # M_t = select(M_{t-1} <= 2^(t-1), M_{t-1}, 0) + X_t
                    mt = m_pools[h].tile([C, HB, S], DT, tag=f"m{h}")
                    nc.vector._custom_dve(
                        LIF_OP, out=mt[:], in0=m_cur[h], in1=xt[:],
                        s0=float(2.0 ** (t - 1)),
                    )
                    m = mt[:]
                m_cur[h] = m

                # y_t = Sign(M_t * tau^t - 1) -> uint8 (saturating cast)
                yt = y_pools[h].tile([C, HB * S], mybir.dt.uint8, tag=f"y{h}")
                nc.scalar.activation(
                    yt[:], m.rearrange("c b s -> c (b s)"),
                    mybir.ActivationFunctionType.Sign,
                    bias=-V_TH, scale=float(2.0 ** (-t)),
                )
                nc.scalar.dma_start(
                    out=y_d[t, b0 : b0 + HB].rearrange("b c s -> c b s"),
                    in_=yt[:].rearrange("c (b s) -> c b s", b=HB),
                )
    nc.finalize()
    return nc


_NC_CACHE = None


def _get_nc():
    global _NC_CACHE
    if _NC_CACHE is None:
        _NC_CACHE = build_kernel()
    return _NC_CACHE


def kernel(x: np.ndarray) -> np.ndarray:
    assert x.shape == (T, B, C, H, W), x.shape
    in_dtype = x.dtype
    xf = np.asarray(x, dtype=np.float32).reshape(T, B, C, S)
    # Pre-scale: X_t = x_t / tau^t (exact powers of two).
    scale = (TAU ** -np.arange(T, dtype=np.float64)).astype(np.float32)
    xs = xf * scale[:, None, None, None]

    nc = _get_nc()
    in_maps = [
        {"x": np.ascontiguousarray(xs[:, k * B_LOC : (k + 1) * B_LOC])}
        for k in range(N_CORES)
    ]
    res = run_bass_kernel_spmd(nc, in_maps, list(range(N_CORES)))
    out = np.concatenate([res.results[k]["y"] for k in range(N_CORES)], axis=1)
    return out.reshape(T, B, C, H, W).astype(in_dtype, copy=False)


if __name__ == "__main__":
    x = np.random.randn(T, B, C, H, W).astype(np.float32)
    y = kernel(x)
    print("out", y.shape, y.dtype, "spike rate", y.mean())


# revision 18
# speedup vs baseline: 1.2070x; 1.2070x over previous
"""LIF (leaky integrate-and-fire) forward scan on 8 Trainium2 NeuronCores.

Reference recurrence (per element, scan over T):
    m_t = v_{t-1} * tau + x_t
    y_t = (m_t - v_th > 0) ? 1.0 : 0.0
    v_t = m_t * (1 - y_t)          # hard reset on spike

x: [T=16, B=32, C=128, H=32, W=32] f32.  Data-parallel over B: each core
gets B_loc=4 batches -> per-step tile [C=128 partitions, B_loc*H*W=4096].

Scaled formulation (tau = 0.5 is a power of two, so rescaling is EXACT
in fp32): host pre-scales X_t = x_t / tau^t; on-chip track M_t = m_t / tau^t:
    M_{t+1} = select(M_t <= 2^t, M_t, 0) + X_{t+1}
    y_t     = Sign(M_t * tau^t - 1)

The whole step update is ONE custom DVE op (registered at import into
concourse's custom-DVE table machinery; the uop program is written into
the per-NEFF DVE table, no firmware change).  The spike extraction is
one ACT op per step whose f32->uint8 output cast saturates Sign's -1
to 0, giving exact {0,1} bytes.  Per-core engine busy: DVE ~65us,
ACT ~60us, DMA ~111us -> the kernel is DMA(HBM)-bound.
"""

import sys

sys.path.insert(0, "/opt/trn_rl_repo")

from contextlib import ExitStack

import numpy as np

import concourse.bass as bass
import concourse.tile as tile
from concourse import bacc, mybir
from concourse.bass_utils import run_bass_kernel_spmd

# Hyperparameters (from the nn.Module)
V_TH = 1.0
TAU = 0.5

# Shapes (hardcoded per problem spec)
T, B, C, H, W = 16, 32, 128, 32, 32
N_CORES = 8
B_LOC = B // N_CORES           # 4 batches per core
S = H * W                      # 1024 spatial sites
FREE = B_LOC * S               # 4096 columns per step tile

DT = mybir.dt.float32

# ---- custom DVE op: out = select(in0 <= s0, in0, 0) + in1 ---------------- #


def _register_lif_op():
    from concourse import dve_ops
    from concourse.dve_spec import Spec, Src0, Src1, C0, Zero, select, lower
    from concourse.dve_uop import DveOpSpec

    name = "LIF_GATE_ADD_ANT"
    for op in dve_ops.OPS:
        if op.name == name:
            return op

    body = select(Src0 <= C0, Src0, Zero) + Src1
    spec = Spec(
        body=body,
        reference=lambda in0, in1, s0, s1, imm2: np.where(in0 <= s0, in0, 0.0)
        + in1,
    )
    row = dve_ops._CUSTOM_DVE_ROW_BASE + len(dve_ops.OPS)
    assert row < 0x20, "custom-DVE row field overflow"
    shas = {
        ver: DveOpSpec(
            name=name, opcode=row, uops=lower(spec, ver=ver), rd1_en=True
        ).sha(ver)
        for ver in ("v3", "v4")
    }
    op = dve_ops.DveOp(name, spec, subdim=False, uops_sha=shas)
    dve_ops.OPS.append(op)
    dve_ops._SUB_OPCODE_FOR_NAME[name] = row
    dve_ops.CUSTOM_DVE_SPECS[name] = spec
    return op


LIF_OP = _register_lif_op()


def build_kernel() -> bass.Bass:
    nc = bacc.Bacc(
        "TRN2", target_bir_lowering=False, debug=False, num_devices=N_CORES
    )
    x_d = nc.dram_tensor("x", [T, B_LOC, C, S], DT, kind="ExternalInput").ap()
    y_d = nc.dram_tensor(
        "y", [T, B_LOC, C, S], mybir.dt.uint8, kind="ExternalOutput"
    ).ap()

    # Const AP for the ACT bias (-1.0); activation bias needs [128,1] SBUF.
    _c = nc.alloc_sbuf_tensor(f"const-float32-{-V_TH}", [128, 1], DT)
    nc.vector.memset(_c.ap(), -V_TH)
    nc.const_aps.aps[(DT, -V_TH)] = _c.ap()
    nc.all_engine_barrier()

    HB = B_LOC // 2  # 2 independent column-half chains (finer pipelining)
    with ExitStack() as ctx:
        tc = ctx.enter_context(tile.TileContext(nc))
        x_pools = [
            ctx.enter_context(tc.tile_pool(name=f"x{h}", bufs=7))
            for h in range(2)
        ]
        m_pools = [
            ctx.enter_context(tc.tile_pool(name=f"m{h}", bufs=3))
            for h in range(2)
        ]
        y_pools = [
            ctx.enter_context(tc.tile_pool(name=f"y{h}", bufs=6))
            for h in range(2)
        ]

        mq_pools = [
            ctx.enter_context(tc.tile_pool(name=f"mq{h}", bufs=2))
            for h in range(2)
        ]

        m_cur = [None, None]
        for t in range(T):
            for h in range(2):
                b0 = h * HB
                xt = x_pools[h].tile([C, HB, S], DT, tag=f"x{h}")
                if t == 0:
                    # Head: two quarter DMAs into the tile so the first
                    # Sign can start as soon as the first quarter lands.
                    for q in range(2):
                        bq = b0 + q * (HB // 2)
                        nc.sync.dma_start(
                            out=xt[:, q * (HB // 2) : (q + 1) * (HB // 2)],
                            in_=x_d[t, bq : bq + HB // 2].rearrange(
                                "b c s -> c b s"
                            ),
                        )
                else:
                    nc.sync.dma_start(
                        out=xt[:],
                        in_=x_d[t, b0 : b0 + HB].rearrange("b c s -> c b s"),
                    )

                if t == T - 1:
                    # Tail: quarter-width chains so the last Sign/y-DMA
                    # pipeline drains sooner after the input stream ends.
                    for q in range(2):
                        mq = mq_pools[h].tile([C, HB // 2, S], DT, tag=f"mq{h}")
                        nc.vector._custom_dve(
                            LIF_OP, out=mq[:],
                            in0=m_cur[h][:, q * (HB // 2) : (q + 1) * (HB // 2)],
                            in1=xt[:, q * (HB // 2) : (q + 1) * (HB // 2)],
                            s0=float(2.0 ** (t - 1)),
                        )
                        yq = y_pools[h].tile(
                            [C, (HB // 2) * S], mybir.dt.uint8, tag=f"y{h}"
                        )
                        nc.scalar.activation(
                            yq[:], mq[:].rearrange("c b s -> c (b s)"),
                            mybir.ActivationFunctionType.Sign,
                            bias=-V_TH, scale=float(2.0 ** (-t)),
                        )
                        bq = b0 + q * (HB // 2)
                        nc.scalar.dma_start(
                            out=y_d[t, bq : bq + HB // 2].rearrange(
                                "b c s -> c b s"
                            ),
                            in_=yq[:].rearrange(
                                "c (b s) -> c b s", b=HB // 2
                            ),
                        )
                    continue

                if t == 0:
                    m = xt[:]
                else:
                    # M_t = select(M_{t-1} <= 2^(t-1), M_{t-1}, 0) + X_t
                    mt = m_pools[h].tile([C, HB, S], DT, tag=f"m{h}")
                    nc.vector._custom_dve(
                        LIF_OP, out=mt[:], in0=m_cur[h], in1=xt[:],
                        s0=float(2.0 ** (t - 1)),
                    )
                    m = mt[:]
                m_cur[h] = m

                # y_t = Sign(M_t * tau^t - 1) -> uint8, quarter-width for
                # a smoother output-DMA stream
                for q in range(2):
                    yq = y_pools[h].tile(
                        [C, (HB // 2) * S], mybir.dt.uint8, tag=f"y{h}"
                    )
                    nc.scalar.activation(
                        yq[:],
                        m[:, q * (HB // 2) : (q + 1) * (HB // 2)].rearrange(
                            "c b s -> c (b s)"
                        ),
                        mybir.ActivationFunctionType.Sign,
                        bias=-V_TH, scale=float(2.0 ** (-t)),
                    )
                    bq = b0 + q * (HB // 2)
                    nc.scalar.dma_start(
                        out=y_d[t, bq : bq + HB // 2].rearrange("b c s -> c b s"),
                        in_=yq[:].rearrange("c (b s) -> c b s", b=HB // 2),
                    )
    nc.finalize()
    return nc


_NC_CACHE = None


def _get_nc():
    global _NC_CACHE
    if _NC_CACHE is None:
        _NC_CACHE = build_kernel()
    return _NC_CACHE


def kernel(x: np.ndarray) -> np.ndarray:
    assert x.shape == (T, B, C, H, W), x.shape
    in_dtype = x.dtype
    xf = np.asarray(x, dtype=np.float32).reshape(T, B, C, S)
    # Pre-scale: X_t = x_t / tau^t (exact powers of two).
    scale = (TAU ** -np.arange(T, dtype=np.float64)).astype(np.float32)
    xs = xf * scale[:, None, None, None]

    nc = _get_nc()
    in_maps = [
        {"x": np.ascontiguousarray(xs[:, k * B_LOC : (k + 1) * B_LOC])}
        for k in range(N_CORES)
    ]
    res = run_bass_kernel_spmd(nc, in_maps, list(range(N_CORES)))
    out = np.concatenate([res.results[k]["y"] for k in range(N_CORES)], axis=1)
    return out.reshape(T, B, C, H, W).astype(in_dtype, copy=False)


if __name__ == "__main__":
    x = np.random.randn(T, B, C, H, W).astype(np.float32)
    y = kernel(x)
    print("out", y.shape, y.dtype, "spike rate", y.mean())
